# revision 17
# baseline (speedup 1.0000x reference)
"""Masked dot-product attention on 8 Trainium2 NeuronCores.

Problem: B=8, S=4096, D=64 fp32; per-batch key-length mask; softmax over keys.

Sharding: sequence-parallel over Q rows. Each core computes a 512-row Q slice
of all 8 batches; per-batch key loops have identical trip counts on every
core -> one SPMD program, perfectly balanced.

Design (per (batch, core) unit, scores kept transposed [k, q]):
  phase 1: k-tiles processed in PAIRS, row-packed on the PE array. The
    contraction dim is D=64, so tile 2p uses array rows 0-63
    (tile_position (0,0)) and tile 2p+1 uses rows 64-127 ((64,0)); the two
    N=512 matmuls run concurrently (second MM retires ~free) -> ~2x
    phase-1 throughput. K is staged even-tiles -> partitions 0-63,
    odd-tiles -> 64-127 (host splits; no extra bytes); Q is DMA'd twice
    (both partition halves).
  exp: pairs alternate between ScalarE (exact ACT exp, PSUM->SBUF bf16)
    and VectorE (Schraudolph: e = bitcast_bf16(i16(round(a*s + b))) in ONE
    tensor_scalar with int16-convert output — the convert is exact
    round-to-nearest; max rel err ~3.3%, washed out by softmax averaging).
    Splitting roughly doubles exp throughput vs ScalarE alone (exp is
    ScalarE-only otherwise). Batches with < MIN_NK_DVE k-tiles (weak
    averaging) stay on the exact ACT path.
  phase 2: psum_o[128, q=512] += V_tile.T @ e_tile. V padded to 128 weight
    cols (FWL) with col 64 = ones, so row 64 accumulates the denominator.
  tail: den staged PSUM->SBUF on ScalarE (custom-DVE ops misread PSUM),
    reciprocal_approx_fast (custom DVE, ~5x faster than iterative divide),
    GpSimd partition-broadcast, DVE multiply, DMA out in [d, q] layout;
    host transposes back.

Hardware facts this build depends on (measured on these cores):
  - The PE HAM clock gate NEVER opens on K=64-contraction matmuls: they
    run at 1.2 GHz (427 ns for N=512) forever. Only K=128 matmuls count
    as "busy", warming the clock to 2.4 GHz (~216 ns) after ~3.5 us. The
    warm-up burst therefore MUST use K=128 weights; phase-2's K=128
    stream keeps the gate open thereafter.
  - Matmul free dim caps at 512 (ISA s3d3_mm_num_elements).
  - DMA queues drain ~serially: the first unit's deps (q slice, K, V of
    the first batch) are issued first; ACT's exp table set is preloaded
    with a dummy activation so the first real exp skips ACT_TABLE_LOAD.

The unit loop is software-pipelined two deep: scores(i+1) and scores(i+2)
sit in the PE queue ahead of phase2(i), so exp(i) latency (either engine)
never stalls the PE. Batches are processed largest-first so the exposed
final tail belongs to the smallest batch; per-batch trip counts are
identical on every core -> one SPMD program, zero imbalance.

Masking is free: host zeroes V rows (incl. ones col) at key positions >=
valid_len, so masked keys add 0 to numerator and denominator; their score
columns hold finite garbage that exp maps to finite values times zero.
"""

import math
from contextlib import ExitStack

import numpy as np

B = 8
S = 4096
D = 64
N_CORES = 8
QB = S // N_CORES  # 512 q rows per core per batch
KT = 128  # k rows per tile
NKMAX = S // KT  # 32
NPMAX = NKMAX // 2  # 16 pairs max
SCALE = 1.0 / math.sqrt(D)
LOG2E = 1.4426950408889634
A16 = 128.0 * LOG2E * SCALE      # Schraudolph bf16 multiplier
B16 = 16250.375                  # tuned bias (round-to-nearest convert)
MIN_NK_DVE = 8                   # batches with fewer k-tiles stay on ACT

_PROGRAM_CACHE: dict = {}


def _build_program(k_tiles):
    import concourse.tile as tile
    from concourse import bacc, mybir

    f32 = mybir.dt.float32
    bf16 = mybir.dt.bfloat16
    i16 = mybir.dt.int16
    nc = bacc.Bacc("TRN2", target_bir_lowering=False, debug=False,
                   enable_asserts=False, num_devices=N_CORES)

    qx = nc.dram_tensor("qx", [D, B * QB], bf16, kind="ExternalInput").ap()
    kxe = nc.dram_tensor("kxe", [B, D, NPMAX * KT], bf16,
                         kind="ExternalInput").ap()
    kxo = nc.dram_tensor("kxo", [B, D, NPMAX * KT], bf16,
                         kind="ExternalInput").ap()
    vx = nc.dram_tensor("vx", [B, KT, NKMAX, KT], bf16,
                        kind="ExternalInput").ap()
    out = nc.dram_tensor("out", [B, D, QB], f32, kind="ExternalOutput").ap()

    order = sorted(range(B), key=lambda x: -k_tiles[x])
    units = []  # (batch, pair_idx)
    for b in order:
        nk = k_tiles[b]
        units.extend((b, p) for p in range((nk + 1) // 2))

    with tile.TileContext(nc) as tc:
        with ExitStack() as ctx:
            q_pool = ctx.enter_context(tc.tile_pool(name="q", bufs=1))
            k_pool = ctx.enter_context(tc.tile_pool(name="k", bufs=2))
            v_pool = ctx.enter_context(tc.tile_pool(name="v", bufs=2))
            e_pool = ctx.enter_context(tc.tile_pool(name="e", bufs=6))
            n_pool = ctx.enter_context(tc.tile_pool(name="n", bufs=2))
            ps_s_pool = ctx.enter_context(
                tc.tile_pool(name="ps_s", bufs=3, space="PSUM"))
            ps_o_pool = ctx.enter_context(
                tc.tile_pool(name="ps_o", bufs=2, space="PSUM"))

            k_sb = {}
            v_sb = {}
            ps_o = {}
            q_all = q_pool.tile([KT, B * QB], bf16)

            def load_batch(b, chunks=1):
                # chunks>1 splits the transfers across more DMA queues so
                # the first tiles land sooner (each dma_start binds to one
                # of 16 queues); subtile deps let scores start per-tile.
                nk = k_tiles[b]
                ne, no = (nk + 1) // 2, nk // 2
                kt_ = k_pool.tile([KT, NPMAX * KT], bf16)
                for dst, src, n in ((slice(0, D), kxe, ne),
                                    (slice(D, KT), kxo, no)):
                    if not n:
                        continue
                    step = max(KT, (n * KT + chunks - 1) // chunks // KT * KT)
                    for c0 in range(0, n * KT, step):
                        c1 = min(c0 + step, n * KT)
                        nc.sync.dma_start(kt_[dst, c0:c1], src[b][:, c0:c1])
                k_sb[b] = kt_
                vt = v_pool.tile([KT, NKMAX * KT], bf16)
                tstep = max(1, (nk + chunks - 1) // chunks)
                for t0 in range(0, nk, tstep):
                    t1 = min(t0 + tstep, nk)
                    nc.sync.dma_start(
                        vt[:, t0 * KT:t1 * KT].rearrange(
                            "p (t c) -> p t c", c=KT),
                        vx[b][:, t0:t1, :])
                v_sb[b] = vt

            def scores(u):
                b, p = u
                nk = k_tiles[b]
                full = 2 * p + 1 < nk
                ps = ps_s_pool.tile([KT, 2 * QB], f32)
                q_lo = q_all[:D, b * QB:(b + 1) * QB]
                q_hi = q_all[D:, b * QB:(b + 1) * QB]
                nc.tensor.matmul(ps[:, :QB],
                                 lhsT=k_sb[b][:D, p * KT:(p + 1) * KT],
                                 rhs=q_lo, start=True, stop=True,
                                 tile_position=(0, 0))
                if full:
                    nc.tensor.matmul(ps[:, QB:],
                                     lhsT=k_sb[b][D:, p * KT:(p + 1) * KT],
                                     rhs=q_hi, start=True, stop=True,
                                     tile_position=(64, 0))
                return ps

            def exp_pair(u, ps, use_dve):
                b, p = u
                nk = k_tiles[b]
                w = QB * (2 if 2 * p + 1 < nk else 1)
                e_sb = e_pool.tile([KT, 2 * QB], bf16)
                if use_dve:
                    nc.vector.tensor_scalar(
                        e_sb[:, :w].bitcast(i16), ps[:, :w], A16, B16,
                        mybir.AluOpType.mult, mybir.AluOpType.add)
                else:
                    nc.scalar.activation(
                        e_sb[:, :w], ps[:, :w],
                        mybir.ActivationFunctionType.Exp, scale=SCALE)
                return e_sb

            def phase2(u, e_sb):
                b, p = u
                nk = k_tiles[b]
                for tl in range(2):
                    kt = 2 * p + tl
                    if kt >= nk:
                        break
                    nc.tensor.matmul(
                        ps_o[b][:],
                        lhsT=v_sb[b][:, kt * KT:(kt + 1) * KT],
                        rhs=e_sb[:, tl * QB:(tl + 1) * QB],
                        start=(kt == 0), stop=(kt == nk - 1),
                        skip_group_check=True)

            def tail(b):
                # reciprocal_approx_fast misreads PSUM operands (the custom
                # DVE uop program expects SBUF): stage the den row first.
                # ACT-only batches (small nk, incl. the last-scheduled one)
                # stage on DVE instead — their ScalarE queue is busy with
                # the final exp while the DVE is idle.
                den_sb = n_pool.tile([1, QB], f32, tag="den_sb", bufs=2)
                if k_tiles[b] < MIN_NK_DVE:
                    nc.vector.tensor_copy(den_sb[:], ps_o[b][D:D + 1, :])
                else:
                    nc.scalar.copy(den_sb[:], ps_o[b][D:D + 1, :])
                r_row = n_pool.tile([1, QB], f32, tag="r_row", bufs=2)
                nc.vector.reciprocal_approx_fast(r_row[:], den_sb[:])
                r_b = n_pool.tile([D, QB], f32, tag="r_b", bufs=2)
                nc.gpsimd.partition_broadcast(r_b[:], r_row[:])
                o_n = n_pool.tile([D, QB], f32, tag="o_n", bufs=2)
                nc.vector.tensor_mul(o_n[:], ps_o[b][:D, :], r_b[:])
                nc.sync.dma_start(out[b], o_n[:])
                del ps_o[b]

            # ---- software-pipelined unit loop ----
            n_units = len(units)
            # engine pattern: alternate ACT/DVE, but ACT-only for small nk
            use_dve = []
            flip = False
            for (b, p) in units:
                if k_tiles[b] < MIN_NK_DVE:
                    use_dve.append(False)
                else:
                    use_dve.append(flip)
                    flip = not flip

            # DMA issue order matters: the queues drain ~serially, so the
            # first unit's dependencies (first batch's q slice + K + V) go
            # first; the rest of q and batch 2 follow.
            b0 = order[0]
            nc.sync.dma_start(q_all[:D, b0 * QB:(b0 + 1) * QB],
                              qx[:, b0 * QB:(b0 + 1) * QB])
            nc.sync.dma_start(q_all[D:, b0 * QB:(b0 + 1) * QB],
                              qx[:, b0 * QB:(b0 + 1) * QB])
            load_batch(b0, chunks=4)
            for bq in order[1:]:
                nc.sync.dma_start(q_all[:D, bq * QB:(bq + 1) * QB],
                                  qx[:, bq * QB:(bq + 1) * QB])
                nc.sync.dma_start(q_all[D:, bq * QB:(bq + 1) * QB],
                                  qx[:, bq * QB:(bq + 1) * QB])
            if len(order) > 1:
                load_batch(order[1])
            next_load = 2

            # HAM warm-up while the first DMAs land. MUST be K=128
            # (full-array contraction): K=64 matmuls never trip the HAM
            # busy window and the PE stays clock-gated at 1.2 GHz.
            wu_sb = q_pool.tile([KT, QB], bf16, tag="warm", bufs=1)
            nc.gpsimd.memset(wu_sb[:], 0.0)
            # preload the exp table set (~1.3us) off the critical path so
            # the first real exp doesn't pay ACT_TABLE_LOAD inline.
            wu_e = q_pool.tile([1, 8], bf16, tag="warm_e", bufs=1)
            nc.scalar.activation(wu_e[:], wu_sb[0:1, :8],
                                 mybir.ActivationFunctionType.Exp, scale=SCALE)
            ps_w = ps_o_pool.tile([KT, QB], f32, tag="ps_o", name="ps_o_t")
            for _ in range(10):
                nc.tensor.matmul(ps_w[:], lhsT=wu_sb[:, :KT],
                                 rhs=wu_sb[:], start=True, stop=True)

            def start_unit(j):
                # batch-transition bookkeeping + issue scores(units[j])
                nonlocal next_load
                nb = units[j][0]
                if nb not in ps_o:
                    ps_o[nb] = ps_o_pool.tile([KT, QB], f32, tag="ps_o",
                                              name="ps_o_t")
                    if next_load < len(order):
                        load_batch(order[next_load])
                        next_load += 1
                return scores(units[j])

            # lead-2 pipeline: the PE queue holds scores for units i+1 and
            # i+2 ahead of phase2(i), so exp(i) latency never stalls the PE.
            from collections import deque
            ps_q = deque()
            ps_q.append(start_unit(0))
            if n_units > 1:
                ps_q.append(start_unit(1))
            for i in range(n_units):
                b, p = units[i]
                e_sb = exp_pair(units[i], ps_q.popleft(), use_dve[i])
                if i + 2 < n_units:
                    ps_q.append(start_unit(i + 2))
                phase2(units[i], e_sb)
                if i + 1 >= n_units or units[i + 1][0] != b:
                    tail(b)

    nc.compile()
    return nc


def _prep_inputs(query, key, value, valid):
    import ml_dtypes

    vclamp = np.clip(valid, 1, S)
    k_tiles = tuple(int(x) for x in np.ceil(vclamp / KT).astype(np.int64))

    kxh = np.ascontiguousarray(key.transpose(0, 2, 1)).astype(
        ml_dtypes.bfloat16)  # [B, D, S]
    kxr = kxh.reshape(B, D, NKMAX, KT)
    kxe_h = np.ascontiguousarray(
        kxr[:, :, 0::2, :].reshape(B, D, NPMAX * KT))
    kxo_h = np.ascontiguousarray(
        kxr[:, :, 1::2, :].reshape(B, D, NPMAX * KT))

    vxh = np.zeros((B, S, KT), dtype=np.float32)  # padded to 128 weight cols
    vxh[:, :, :D] = value
    vxh[:, :, D] = 1.0
    for b in range(B):
        vxh[b, vclamp[b]:, :] = 0.0  # masked keys contribute nothing
    vxt = np.ascontiguousarray(
        vxh.reshape(B, NKMAX, KT, KT).transpose(0, 2, 1, 3)
    ).astype(ml_dtypes.bfloat16)
    qt = query.transpose(0, 2, 1)  # [B, D, S]

    in_maps = []
    for c in range(N_CORES):
        qxh = np.ascontiguousarray(
            qt[:, :, c * QB:(c + 1) * QB].transpose(1, 0, 2)
        ).reshape(D, B * QB).astype(ml_dtypes.bfloat16)
        in_maps.append({"qx": qxh, "kxe": kxe_h, "kxo": kxo_h, "vx": vxt})
    return k_tiles, in_maps


def kernel(query, key, value, valid_len):
    from concourse.bass_utils import run_bass_kernel_spmd

    query = np.ascontiguousarray(query, dtype=np.float32)
    key = np.ascontiguousarray(key, dtype=np.float32)
    value = np.ascontiguousarray(value, dtype=np.float32)
    valid = np.asarray(valid_len).astype(np.int64)
    assert query.shape == (B, S, D) and key.shape == (B, S, D)
    assert value.shape == (B, S, D) and valid.shape == (B,)

    k_tiles, in_maps = _prep_inputs(query, key, value, valid)

    nc = _PROGRAM_CACHE.get(k_tiles)
    if nc is None:
        nc = _build_program(k_tiles)
        _PROGRAM_CACHE[k_tiles] = nc

    res = run_bass_kernel_spmd(nc, in_maps, core_ids=list(range(N_CORES)))

    full = np.empty((B, S, D), dtype=np.float32)
    for c in range(N_CORES):
        # out is [B, D, QB]; transpose back
        full[:, c * QB:(c + 1) * QB, :] = res.results[c]["out"].transpose(0, 2, 1)

    # valid_len == 0 never occurs per the spec (randint >= 1), but the
    # reference would produce uniform attention there; match it exactly.
    if np.any(valid < 1):
        for b in np.nonzero(valid < 1)[0]:
            sc = (query[b] @ key[b].T) * SCALE - 1.0e6
            a = np.exp(sc - sc.max(axis=-1, keepdims=True))
            a /= a.sum(axis=-1, keepdims=True)
            full[b] = a @ value[b]

    return full


# revision 18
# speedup vs baseline: 1.0076x; 1.0076x over previous
"""Masked dot-product attention on 8 Trainium2 NeuronCores.

Problem: B=8, S=4096, D=64 fp32; per-batch key-length mask; softmax over keys.

Sharding: sequence-parallel over Q rows. Each core computes a 512-row Q slice
of all 8 batches; per-batch key loops have identical trip counts on every
core -> one SPMD program, perfectly balanced.

Design (per (batch, core) unit, scores kept transposed [k, q]):
  phase 1: k-tiles processed in PAIRS, row-packed on the PE array. The
    contraction dim is D=64, so tile 2p uses array rows 0-63
    (tile_position (0,0)) and tile 2p+1 uses rows 64-127 ((64,0)); the two
    N=512 matmuls run concurrently (second MM retires ~free) -> ~2x
    phase-1 throughput. K is staged even-tiles -> partitions 0-63,
    odd-tiles -> 64-127 (host splits; no extra bytes); Q is DMA'd twice
    (both partition halves).
  exp: pairs alternate between ScalarE (exact ACT exp, PSUM->SBUF bf16)
    and VectorE (Schraudolph: e = bitcast_bf16(i16(round(a*s + b))) in ONE
    tensor_scalar with int16-convert output — the convert is exact
    round-to-nearest; max rel err ~3.3%, washed out by softmax averaging).
    Splitting roughly doubles exp throughput vs ScalarE alone (exp is
    ScalarE-only otherwise). Batches with < MIN_NK_DVE k-tiles (weak
    averaging) stay on the exact ACT path.
  phase 2: psum_o[128, q=512] += V_tile.T @ e_tile. V padded to 128 weight
    cols (FWL) with col 64 = ones, so row 64 accumulates the denominator.
  tail: den staged PSUM->SBUF on ScalarE (custom-DVE ops misread PSUM),
    reciprocal_approx_fast (custom DVE, ~5x faster than iterative divide),
    GpSimd partition-broadcast, DVE multiply, DMA out in [d, q] layout;
    host transposes back.

Hardware facts this build depends on (measured on these cores):
  - The PE HAM clock gate NEVER opens on K=64-contraction matmuls: they
    run at 1.2 GHz (427 ns for N=512) forever. Only K=128 matmuls count
    as "busy", warming the clock to 2.4 GHz (~216 ns) after ~3.5 us. The
    warm-up burst therefore MUST use K=128 weights; phase-2's K=128
    stream keeps the gate open thereafter.
  - Matmul free dim caps at 512 (ISA s3d3_mm_num_elements).
  - DMA queues drain ~serially: the first unit's deps (q slice, K, V of
    the first batch) are issued first; ACT's exp table set is preloaded
    with a dummy activation so the first real exp skips ACT_TABLE_LOAD.

The unit loop is software-pipelined two deep: scores(i+1) and scores(i+2)
sit in the PE queue ahead of phase2(i), so exp(i) latency (either engine)
never stalls the PE. Batches are processed largest-first so the exposed
final tail belongs to the smallest batch; per-batch trip counts are
identical on every core -> one SPMD program, zero imbalance.

Masking is free: host zeroes V rows (incl. ones col) at key positions >=
valid_len, so masked keys add 0 to numerator and denominator; their score
columns hold finite garbage that exp maps to finite values times zero.
"""

import math
from contextlib import ExitStack

import numpy as np

B = 8
S = 4096
D = 64
N_CORES = 8
QB = S // N_CORES  # 512 q rows per core per batch
KT = 128  # k rows per tile
NKMAX = S // KT  # 32
NPMAX = NKMAX // 2  # 16 pairs max
SCALE = 1.0 / math.sqrt(D)
LOG2E = 1.4426950408889634
A16 = 128.0 * LOG2E * SCALE      # Schraudolph bf16 multiplier
B16 = 16250.375                  # tuned bias (round-to-nearest convert)
MIN_NK_DVE = 8                   # batches with fewer k-tiles stay on ACT

_PROGRAM_CACHE: dict = {}


def _build_program(k_tiles):
    import concourse.tile as tile
    from concourse import bacc, mybir

    f32 = mybir.dt.float32
    bf16 = mybir.dt.bfloat16
    i16 = mybir.dt.int16
    nc = bacc.Bacc("TRN2", target_bir_lowering=False, debug=False,
                   enable_asserts=False, num_devices=N_CORES)

    qx = nc.dram_tensor("qx", [D, B * QB], bf16, kind="ExternalInput").ap()
    kxe = nc.dram_tensor("kxe", [B, D, NPMAX * KT], bf16,
                         kind="ExternalInput").ap()
    kxo = nc.dram_tensor("kxo", [B, D, NPMAX * KT], bf16,
                         kind="ExternalInput").ap()
    vx = nc.dram_tensor("vx", [B, KT, NKMAX, KT], bf16,
                        kind="ExternalInput").ap()
    out = nc.dram_tensor("out", [B, D, QB], f32, kind="ExternalOutput").ap()

    order = sorted(range(B), key=lambda x: -k_tiles[x])
    units = []  # (batch, pair_idx)
    for b in order:
        nk = k_tiles[b]
        units.extend((b, p) for p in range((nk + 1) // 2))

    with tile.TileContext(nc) as tc:
        with ExitStack() as ctx:
            q_pool = ctx.enter_context(tc.tile_pool(name="q", bufs=1))
            k_pool = ctx.enter_context(tc.tile_pool(name="k", bufs=2))
            v_pool = ctx.enter_context(tc.tile_pool(name="v", bufs=2))
            e_pool = ctx.enter_context(tc.tile_pool(name="e", bufs=6))
            n_pool = ctx.enter_context(tc.tile_pool(name="n", bufs=2))
            ps_s_pool = ctx.enter_context(
                tc.tile_pool(name="ps_s", bufs=3, space="PSUM"))
            ps_o_pool = ctx.enter_context(
                tc.tile_pool(name="ps_o", bufs=2, space="PSUM"))

            k_sb = {}
            v_sb = {}
            ps_o = {}
            q_all = q_pool.tile([KT, B * QB], bf16)

            def load_batch(b, chunks=1):
                # chunks>1 splits the transfers across more DMA queues so
                # the first tiles land sooner (each dma_start binds to one
                # of 16 queues); subtile deps let scores start per-tile.
                nk = k_tiles[b]
                ne, no = (nk + 1) // 2, nk // 2
                kt_ = k_pool.tile([KT, NPMAX * KT], bf16)
                for dst, src, n in ((slice(0, D), kxe, ne),
                                    (slice(D, KT), kxo, no)):
                    if not n:
                        continue
                    step = max(KT, (n * KT + chunks - 1) // chunks // KT * KT)
                    for c0 in range(0, n * KT, step):
                        c1 = min(c0 + step, n * KT)
                        nc.sync.dma_start(kt_[dst, c0:c1], src[b][:, c0:c1])
                k_sb[b] = kt_
                vt = v_pool.tile([KT, NKMAX * KT], bf16)
                tstep = max(1, (nk + chunks - 1) // chunks)
                for t0 in range(0, nk, tstep):
                    t1 = min(t0 + tstep, nk)
                    nc.sync.dma_start(
                        vt[:, t0 * KT:t1 * KT].rearrange(
                            "p (t c) -> p t c", c=KT),
                        vx[b][:, t0:t1, :])
                v_sb[b] = vt

            def scores(u):
                b, p = u
                nk = k_tiles[b]
                full = 2 * p + 1 < nk
                ps = ps_s_pool.tile([KT, 2 * QB], f32)
                q_lo = q_all[:D, b * QB:(b + 1) * QB]
                q_hi = q_all[D:, b * QB:(b + 1) * QB]
                nc.tensor.matmul(ps[:, :QB],
                                 lhsT=k_sb[b][:D, p * KT:(p + 1) * KT],
                                 rhs=q_lo, start=True, stop=True,
                                 tile_position=(0, 0))
                if full:
                    nc.tensor.matmul(ps[:, QB:],
                                     lhsT=k_sb[b][D:, p * KT:(p + 1) * KT],
                                     rhs=q_hi, start=True, stop=True,
                                     tile_position=(64, 0))
                return ps

            def exp_pair(u, ps, use_dve):
                b, p = u
                nk = k_tiles[b]
                w = QB * (2 if 2 * p + 1 < nk else 1)
                e_sb = e_pool.tile([KT, 2 * QB], bf16)
                if use_dve:
                    nc.vector.tensor_scalar(
                        e_sb[:, :w].bitcast(i16), ps[:, :w], A16, B16,
                        mybir.AluOpType.mult, mybir.AluOpType.add)
                else:
                    nc.scalar.activation(
                        e_sb[:, :w], ps[:, :w],
                        mybir.ActivationFunctionType.Exp, scale=SCALE)
                return e_sb

            def phase2(u, e_sb):
                b, p = u
                nk = k_tiles[b]
                for tl in range(2):
                    kt = 2 * p + tl
                    if kt >= nk:
                        break
                    nc.tensor.matmul(
                        ps_o[b][:],
                        lhsT=v_sb[b][:, kt * KT:(kt + 1) * KT],
                        rhs=e_sb[:, tl * QB:(tl + 1) * QB],
                        start=(kt == 0), stop=(kt == nk - 1),
                        skip_group_check=True)

            def tail(b):
                # reciprocal_approx_fast misreads PSUM operands (the custom
                # DVE uop program expects SBUF): stage the den row first.
                # ACT-only batches (small nk, incl. the last-scheduled one)
                # stage on DVE instead — their ScalarE queue is busy with
                # the final exp while the DVE is idle.
                den_sb = n_pool.tile([1, QB], f32, tag="den_sb", bufs=2)
                if k_tiles[b] < MIN_NK_DVE:
                    nc.vector.tensor_copy(den_sb[:], ps_o[b][D:D + 1, :])
                else:
                    nc.scalar.copy(den_sb[:], ps_o[b][D:D + 1, :])
                r_row = n_pool.tile([1, QB], f32, tag="r_row", bufs=2)
                nc.vector.reciprocal_approx_fast(r_row[:], den_sb[:])
                r_b = n_pool.tile([D, QB], f32, tag="r_b", bufs=2)
                nc.gpsimd.partition_broadcast(r_b[:], r_row[:])
                o_n = n_pool.tile([D, QB], f32, tag="o_n", bufs=2)
                nc.vector.tensor_mul(o_n[:], ps_o[b][:D, :], r_b[:])
                nc.sync.dma_start(out[b], o_n[:])
                del ps_o[b]

            # ---- software-pipelined unit loop ----
            n_units = len(units)
            # engine pattern: alternate ACT/DVE, but ACT-only for small nk
            use_dve = []
            flip = False
            for (b, p) in units:
                if k_tiles[b] < MIN_NK_DVE:
                    use_dve.append(False)
                else:
                    use_dve.append(flip)
                    flip = not flip

            # DMA issue order matters: the queues drain ~serially, so the
            # first unit's dependencies (first batch's q slice + K + V) go
            # first; the rest of q and batch 2 follow.
            b0 = order[0]
            nc.sync.dma_start(q_all[:D, b0 * QB:(b0 + 1) * QB],
                              qx[:, b0 * QB:(b0 + 1) * QB])
            nc.sync.dma_start(q_all[D:, b0 * QB:(b0 + 1) * QB],
                              qx[:, b0 * QB:(b0 + 1) * QB])
            load_batch(b0, chunks=4)
            for bq in order[1:]:
                nc.sync.dma_start(q_all[:D, bq * QB:(bq + 1) * QB],
                                  qx[:, bq * QB:(bq + 1) * QB])
                nc.sync.dma_start(q_all[D:, bq * QB:(bq + 1) * QB],
                                  qx[:, bq * QB:(bq + 1) * QB])
            if len(order) > 1:
                load_batch(order[1])
            next_load = 2

            # HAM warm-up while the first DMAs land. MUST be K=128
            # (full-array contraction): K=64 matmuls never trip the HAM
            # busy window and the PE stays clock-gated at 1.2 GHz.
            wu_sb = q_pool.tile([KT, QB], bf16, tag="warm", bufs=1)
            nc.gpsimd.memset(wu_sb[:], 0.0)
            # preload the exp table set (~1.3us) off the critical path so
            # the first real exp doesn't pay ACT_TABLE_LOAD inline.
            wu_e = q_pool.tile([1, 8], bf16, tag="warm_e", bufs=1)
            nc.scalar.activation(wu_e[:], wu_sb[0:1, :8],
                                 mybir.ActivationFunctionType.Exp, scale=SCALE)
            ps_w = ps_o_pool.tile([KT, QB], f32, tag="ps_o", name="ps_o_t")
            for _ in range(12):
                nc.tensor.matmul(ps_w[:], lhsT=wu_sb[:, :KT],
                                 rhs=wu_sb[:], start=True, stop=True)

            def start_unit(j):
                # batch-transition bookkeeping + issue scores(units[j])
                nonlocal next_load
                nb = units[j][0]
                if nb not in ps_o:
                    ps_o[nb] = ps_o_pool.tile([KT, QB], f32, tag="ps_o",
                                              name="ps_o_t")
                    if next_load < len(order):
                        load_batch(order[next_load])
                        next_load += 1
                return scores(units[j])

            # lead-2 pipeline: the PE queue holds scores for units i+1 and
            # i+2 ahead of phase2(i), so exp(i) latency never stalls the PE.
            from collections import deque
            ps_q = deque()
            ps_q.append(start_unit(0))
            if n_units > 1:
                ps_q.append(start_unit(1))
            for i in range(n_units):
                b, p = units[i]
                e_sb = exp_pair(units[i], ps_q.popleft(), use_dve[i])
                if i + 2 < n_units:
                    ps_q.append(start_unit(i + 2))
                phase2(units[i], e_sb)
                if i + 1 >= n_units or units[i + 1][0] != b:
                    tail(b)

    nc.compile()
    return nc


def _prep_inputs(query, key, value, valid):
    import ml_dtypes

    vclamp = np.clip(valid, 1, S)
    k_tiles = tuple(int(x) for x in np.ceil(vclamp / KT).astype(np.int64))

    kxh = np.ascontiguousarray(key.transpose(0, 2, 1)).astype(
        ml_dtypes.bfloat16)  # [B, D, S]
    kxr = kxh.reshape(B, D, NKMAX, KT)
    kxe_h = np.ascontiguousarray(
        kxr[:, :, 0::2, :].reshape(B, D, NPMAX * KT))
    kxo_h = np.ascontiguousarray(
        kxr[:, :, 1::2, :].reshape(B, D, NPMAX * KT))

    vxh = np.zeros((B, S, KT), dtype=np.float32)  # padded to 128 weight cols
    vxh[:, :, :D] = value
    vxh[:, :, D] = 1.0
    for b in range(B):
        vxh[b, vclamp[b]:, :] = 0.0  # masked keys contribute nothing
    vxt = np.ascontiguousarray(
        vxh.reshape(B, NKMAX, KT, KT).transpose(0, 2, 1, 3)
    ).astype(ml_dtypes.bfloat16)
    qt = query.transpose(0, 2, 1)  # [B, D, S]

    in_maps = []
    for c in range(N_CORES):
        qxh = np.ascontiguousarray(
            qt[:, :, c * QB:(c + 1) * QB].transpose(1, 0, 2)
        ).reshape(D, B * QB).astype(ml_dtypes.bfloat16)
        in_maps.append({"qx": qxh, "kxe": kxe_h, "kxo": kxo_h, "vx": vxt})
    return k_tiles, in_maps


def kernel(query, key, value, valid_len):
    from concourse.bass_utils import run_bass_kernel_spmd

    query = np.ascontiguousarray(query, dtype=np.float32)
    key = np.ascontiguousarray(key, dtype=np.float32)
    value = np.ascontiguousarray(value, dtype=np.float32)
    valid = np.asarray(valid_len).astype(np.int64)
    assert query.shape == (B, S, D) and key.shape == (B, S, D)
    assert value.shape == (B, S, D) and valid.shape == (B,)

    k_tiles, in_maps = _prep_inputs(query, key, value, valid)

    nc = _PROGRAM_CACHE.get(k_tiles)
    if nc is None:
        nc = _build_program(k_tiles)
        _PROGRAM_CACHE[k_tiles] = nc

    res = run_bass_kernel_spmd(nc, in_maps, core_ids=list(range(N_CORES)))

    full = np.empty((B, S, D), dtype=np.float32)
    for c in range(N_CORES):
        # out is [B, D, QB]; transpose back
        full[:, c * QB:(c + 1) * QB, :] = res.results[c]["out"].transpose(0, 2, 1)

    # valid_len == 0 never occurs per the spec (randint >= 1), but the
    # reference would produce uniform attention there; match it exactly.
    if np.any(valid < 1):
        for b in np.nonzero(valid < 1)[0]:
            sc = (query[b] @ key[b].T) * SCALE - 1.0e6
            a = np.exp(sc - sc.max(axis=-1, keepdims=True))
            a /= a.sum(axis=-1, keepdims=True)
            full[b] = a @ value[b]

    return full
